# revision 45
# baseline (speedup 1.0000x reference)
"""GCN graph classification on 8 Trainium2 NeuronCores (Bass/Tile).

Strategy (src-partitioned, SBUF-local gather + ReduceScatter):
  - Nodes are dealt across 8 cores snake-wise by degree; within each core,
    nodes are packed into 98 blocks of 128 slots by a greedy 8-dim balancer
    so that every (src core, dst block) edge-bucket count is nearly equal
    across cores (SPMD-uniform schedule with ~5% padding).
  - Layer 0 collapses to an outer product (input features are all-ones):
    x1 = relu(a * W0 + b0) with a = dinv * segsum(dinv[src]) on host.
  - Each conv layer: y = dinv * (x @ W) computed per-core into an SBUF tile
    (no AllGather, no DRAM table). Messages are produced by SBUF-source
    non-transpose dma_gather (direct InstDMAGatherAnt; the ucode supports
    src_is_sbuf without transpose) on 4 SWDGE queues - ~2.4ns/token vs
    ~7ns/token for HBM-source gathers.
  - Tokens are the core's OUT-edges sorted by (RS-chunk, dst core, dst
    block). One-hot selection matmuls (sel built per (chunk128, block) pair
    on DVE) accumulate z-partials for all 784 global dst blocks in PSUM;
    partials are evicted bf16 to per-chunk DRAM tensors and combined with a
    chunked ReduceScatter(add). The dst core then applies
    x' = relu(dinv * z + b).
  - Mean-pooling per graph via selection matmuls + small AllReduce; the
    classifier head and log_softmax run on-chip.
"""
import sys

sys.path.insert(0, "/opt/trn_rl_repo")

import numpy as np
import ml_dtypes

import concourse.bass as bass
import concourse.bacc as bacc
import concourse.mybir as mybir
import concourse.tile as tile
from concourse.bass_utils import run_bass_kernel_spmd

# problem constants (hardcoded per spec)
N = 100000
E = 1600000
G = 512
H = 128
C = 10
NC = 8
NB = 98                # blocks per core
S = NB * 128           # node slots per core = 12544
NPAD = NC * S          # padded node rows = 100352
NKCH = 4               # ReduceScatter chunks
CHK_BLKS = [25, 25, 25, 23]
CHK_J0 = [0, 25, 50, 75]
CHK_ROWS = [b * 128 for b in CHK_BLKS]   # per-core rows per chunk
GMAX = 2560            # max tokens per gather

F32 = mybir.dt.float32
BF16 = mybir.dt.bfloat16
I16 = mybir.dt.int16
NP_BF16 = ml_dtypes.bfloat16
TDT = BF16
NP_TDT = NP_BF16


def preprocess(edge_index, batch):
    """Host-side graph preprocessing. Returns the (SPMD-uniform) schedule and
    per-core data arrays."""
    edge_index = np.asarray(edge_index, dtype=np.int64)
    batch = np.asarray(batch, dtype=np.int64)

    loop = np.arange(N, dtype=np.int64)
    lsrc = np.concatenate([edge_index[0], loop])
    ldst = np.concatenate([edge_index[1], loop])

    deg = np.bincount(ldst, minlength=N).astype(np.float64)
    dinv = np.where(deg > 0, 1.0 / np.sqrt(deg), 0.0)
    csum = np.bincount(ldst, weights=dinv[lsrc], minlength=N)
    a = (dinv * csum).astype(np.float32)
    dinv32 = dinv.astype(np.float32)

    # token stream covers real edges only; the self-loop contribution
    # y[v] is added locally at the dst core during the x-update
    esrc = edge_index[0]
    edst = edge_index[1]
    EE = esrc.shape[0]

    # ---- node -> core: snake deal by total degree ----
    odeg = np.bincount(esrc, minlength=N)
    tdeg = odeg + np.bincount(edst, minlength=N)
    order = np.argsort(-tdeg, kind="stable")
    pos = np.arange(N)
    p16 = pos % 16
    core_r = np.where(p16 < 8, p16, 15 - p16)
    core = np.empty(N, dtype=np.int64)
    core[order] = core_r

    # ---- in-profile per node: in-edges by src core ----
    M = np.bincount(edst * NC + core[esrc], minlength=N * NC).reshape(N, NC)

    # ---- block assignment within each core: greedy 8-dim balance ----
    jloc = np.empty(N, dtype=np.int64)
    P = np.empty(N, dtype=np.int64)
    for c in range(NC):
        nodes = np.where(core == c)[0]
        prof = M[nodes].astype(np.float64)
        o = np.argsort(-prof.sum(1), kind="stable")
        nodes = nodes[o]
        prof = prof[o]
        Sb = np.zeros((NB, NC))
        cnt = np.zeros(NB, dtype=np.int64)
        jj = np.empty(len(nodes), dtype=np.int64)
        full = np.zeros(NB, dtype=bool)
        for i in range(len(nodes)):
            cand = (Sb + prof[i]).max(axis=1) + cnt * 1e-3
            cand[full] = np.inf
            b = int(np.argmin(cand))
            jj[i] = b
            Sb[b] += prof[i]
            cnt[b] += 1
            if cnt[b] >= 128:
                full[b] = True
        jloc[nodes] = jj
        # slots within block in assignment order
        pp = np.zeros(len(nodes), dtype=np.int64)
        seen = np.zeros(NB, dtype=np.int64)
        for i in range(len(nodes)):
            pp[i] = seen[jj[i]]
            seen[jj[i]] += 1
        P[nodes] = pp
    slot = jloc * 128 + P
    assert P.max() < 128

    # ---- per-slot arrays [NC, 128, NB] ----
    dinv_sl = np.zeros((NC, S), dtype=np.float32)
    a_sl = np.zeros((NC, S), dtype=np.float32)
    batc_sl = np.full((NC, S), -1.0, dtype=np.float32)
    dinv_sl[core, slot] = dinv32
    a_sl[core, slot] = a
    batc_sl[core, slot] = batch.astype(np.float32)

    def to_pj(x):  # [NC, S] -> [NC, 128, NB]
        return np.ascontiguousarray(x.reshape(NC, NB, 128).transpose(0, 2, 1))

    dinv_pj = to_pj(dinv_sl)
    a_pj = to_pj(a_sl)
    batc_pj = to_pj(batc_sl)

    # ---- global block order: (chunk k, dst core d, local block j) ----
    blocks_order = []           # list of (k, d, j)
    block_rank = np.full((NC, NB), -1, dtype=np.int64)
    for k in range(NKCH):
        for d in range(NC):
            for j in range(CHK_J0[k], CHK_J0[k] + CHK_BLKS[k]):
                block_rank[d, j] = len(blocks_order)
                blocks_order.append((k, d, j))
    NBLK_G = len(blocks_order)  # 784

    # ---- edge buckets ----
    sc = core[esrc]
    Be = block_rank[core[edst], jloc[edst]]
    cnt_cb = np.bincount(sc * NBLK_G + Be, minlength=NC * NBLK_G).reshape(
        NC, NBLK_G
    )
    L = cnt_cb.max(axis=0)      # SPMD-uniform segment lengths

    # ---- stream layout ----
    seg0 = np.zeros(NBLK_G, dtype=np.int64)
    chunk_spans = []            # (k, tok0, ntok_padded, real_end)
    tok = 0
    for k in range(NKCH):
        t0 = tok
        for d in range(NC):
            for j in range(CHK_J0[k], CHK_J0[k] + CHK_BLKS[k]):
                B = block_rank[d, j]
                seg0[B] = tok
                tok += L[B]
        real_end = tok
        tok = ((tok + 127) // 128) * 128
        chunk_spans.append((k, t0, tok - t0, real_end))
    TOKP = tok
    assert TOKP % 128 == 0

    # ---- place edges ----
    okey = sc * NBLK_G + Be
    ordk = np.argsort(okey, kind="stable")
    skey = okey[ordk]
    first = np.searchsorted(skey, skey)
    rank = np.arange(EE) - first
    p_stream = seg0[Be[ordk]] + rank

    gidx = np.zeros((NC, TOKP), dtype=np.int16)
    dlocP = np.full((NC, TOKP), -1, dtype=np.int64)
    gidx[sc[ordk], p_stream] = slot[esrc[ordk]].astype(np.int16)
    dlocP[sc[ordk], p_stream] = P[edst[ordk]]

    # chunk-tail padding gets idx=-1 (trailing in its gather -> descs skipped)
    for (k, t0, ntok, real_end) in chunk_spans:
        gidx[:, real_end:t0 + ntok] = -1

    # pos -> block id (or -1 for padding)
    posB = np.full(TOKP, -1, dtype=np.int64)
    for B in range(NBLK_G):
        posB[seg0[B]:seg0[B] + L[B]] = B

    # ---- gathers ----
    gathers = []                # (k, tok0, ntok)
    for (k, t0, ntok, _) in chunk_spans:
        off = t0
        while off < t0 + ntok:
            n = min(GMAX, t0 + ntok - off)
            gathers.append((k, off, n))
            off += n

    # ---- pairs (chunk128 x block) + matmul/evict schedule ----
    # pair list per gather, in stream order
    pairs = []                  # (s, B)
    gather_pair0 = []
    for (k, gt0, gn) in gathers:
        gather_pair0.append(len(pairs))
        for s in range(gt0 // 128, (gt0 + gn) // 128):
            bs = posB[s * 128:(s + 1) * 128]
            seen = []
            for b in bs:
                if b >= 0 and (not seen or seen[-1] != b):
                    assert b not in seen[:-1]
                    seen.append(int(b))
            for b in seen:
                pairs.append((s, b))
    gather_pair0.append(len(pairs))
    NPAIR = len(pairs)

    # first/last flags per block
    pfirst = np.zeros(NPAIR, dtype=bool)
    plast = np.zeros(NPAIR, dtype=bool)
    seenB = set()
    for i, (s, b) in enumerate(pairs):
        if b not in seenB:
            pfirst[i] = True
            seenB.add(b)
    seenB = set()
    for i in range(NPAIR - 1, -1, -1):
        b = pairs[i][1]
        if b not in seenB:
            plast[i] = True
            seenB.add(b)

    # dloc pair columns [NC, 128, NPAIR]
    dloc_cols = np.full((NC, 128, NPAIR), -1.0, dtype=np.float32)
    for i, (s, b) in enumerate(pairs):
        lanes = posB[s * 128:(s + 1) * 128] == b
        dloc_cols[:, lanes, i] = dlocP[:, s * 128:(s + 1) * 128][:, lanes]
    dloc_dev = np.ascontiguousarray(dloc_cols).astype(NP_TDT)

    # eviction row base per block in its z_part_k tensor
    ev_row = np.zeros(NBLK_G, dtype=np.int64)
    ev_k = np.zeros(NBLK_G, dtype=np.int64)
    for B, (k, d, j) in enumerate(blocks_order):
        ev_k[B] = k
        ev_row[B] = d * CHK_ROWS[k] + (j - CHK_J0[k]) * 128

    # idx device layout: [128, TOKP//16] int16 wrap-16, replicated x8
    g16 = np.ascontiguousarray(gidx.reshape(NC, TOKP // 16, 16).transpose(0, 2, 1))
    gidx_dev = np.tile(g16, (1, 8, 1))

    cntg = np.bincount(batch, minlength=G).astype(np.float32)
    invcnt = (1.0 / np.maximum(cntg, 1.0)).reshape(4, 128).T.copy()

    import os
    if os.environ.get("KGCN_VERBOSE"):
        print(f"[preprocess] TOKP={TOKP} real={EE // NC} "
              f"pad={(TOKP - cnt_cb.sum(1).max()) / TOKP:.3f} "
              f"npair={NPAIR} ngath={len(gathers)}")
    sched = {
        "TOKP": TOKP,
        "NPAIR": NPAIR,
        "gathers": gathers,
        "gather_pair0": gather_pair0,
        "pairs": pairs,
        "pfirst": pfirst,
        "plast": plast,
        "ev_row": ev_row,
        "ev_k": ev_k,
        "chunk_spans": chunk_spans,
    }
    percore = {
        "gidx": gidx_dev,
        "dloc": dloc_dev,
        "dinv_pj": dinv_pj,
        "a_pj": a_pj,
        "batc_pj": batc_pj,
    }
    return sched, percore, invcnt


def sbuf_gather_notrans(nc, out_ap, in_ap, idxs_ap, num_idxs, elem_size,
                        queue_num):
    """dma_gather with SBUF source and transpose=False. The Q7 ucode supports
    this combination (normal [tok%128, tok//128, elem] output layout); only
    the bass-level wrapper restricts SBUF sources to transpose mode, so build
    the instruction directly."""
    gps = nc.gpsimd
    _in_ap = [gps.lower_ap(in_ap)]
    _idxs_ap = gps.lower_ap(idxs_ap)
    _out_ap = gps.lower_ap(out_ap)
    return gps.add_instruction(
        mybir.InstDMAGatherAnt(
            name=nc.get_next_instruction_name(),
            ins=[*_in_ap, _idxs_ap, gps.lower_val_access(gps.to_reg(num_idxs))],
            outs=[_out_ap],
            transpose=False,
            num_idxs=num_idxs,
            elem_size=elem_size,
            stride_bytes_256=0,
            gen_mode=0,
            single_packet=False,
            queue_num=queue_num,
            sbuf_tokens_per_rank=128,
            sbuf_free_dim_per_rank=elem_size * 2,   # bytes (bf16)
            sbuf_free_dim_pad_per_rank=0,
            sbuf_byte_offset=0,
        )
    )


def build_program(sched):
    import os
    NLAYER = int(os.environ.get("KGCN_NLAYER", "2"))
    NO_MM = bool(int(os.environ.get("KGCN_NO_MM", "0")))
    NO_SEL = bool(int(os.environ.get("KGCN_NO_SEL", "0")))
    NO_RS = bool(int(os.environ.get("KGCN_NO_RS", "0")))
    TOKP = sched["TOKP"]
    NPAIR = sched["NPAIR"]
    gathers = sched["gathers"]
    gather_pair0 = sched["gather_pair0"]
    pairs = sched["pairs"]
    pfirst = sched["pfirst"]
    plast = sched["plast"]
    ev_row = sched["ev_row"]
    ev_k = sched["ev_k"]

    nc = bacc.Bacc(
        "TRN2",
        target_bir_lowering=False,
        debug=False,
        num_devices=NC,
        num_swdge_queues=4,
    )

    din = {}
    din["gidx"] = nc.dram_tensor("gidx", [128, TOKP // 16], I16, kind="ExternalInput")
    din["dloc"] = nc.dram_tensor("dloc", [128, NPAIR], TDT, kind="ExternalInput")
    din["dinv"] = nc.dram_tensor("dinv", [128, NB], F32, kind="ExternalInput")
    din["acol"] = nc.dram_tensor("acol", [128, NB], F32, kind="ExternalInput")
    din["batchf"] = nc.dram_tensor("batchf", [128, NB], F32, kind="ExternalInput")
    din["W1"] = nc.dram_tensor("W1", [H, H], TDT, kind="ExternalInput")
    din["W2"] = nc.dram_tensor("W2", [H, H], TDT, kind="ExternalInput")
    din["Wp"] = nc.dram_tensor("Wp", [H, C], F32, kind="ExternalInput")
    din["W0r"] = nc.dram_tensor("W0r", [128, H], F32, kind="ExternalInput")
    din["b0r"] = nc.dram_tensor("b0r", [128, H], F32, kind="ExternalInput")
    din["b1r"] = nc.dram_tensor("b1r", [128, H], F32, kind="ExternalInput")
    din["b2r"] = nc.dram_tensor("b2r", [128, H], F32, kind="ExternalInput")
    din["bpr"] = nc.dram_tensor("bpr", [128, C], F32, kind="ExternalInput")
    din["ident"] = nc.dram_tensor("ident", [128, 128], F32, kind="ExternalInput")
    din["identt"] = nc.dram_tensor("identt", [128, 128], TDT, kind="ExternalInput")
    din["iotar"] = nc.dram_tensor("iotar", [128, 128], TDT, kind="ExternalInput")
    din["giota"] = nc.dram_tensor("giota", [128, G], F32, kind="ExternalInput")
    din["invc"] = nc.dram_tensor("invc", [128, 4], F32, kind="ExternalInput")
    out = nc.dram_tensor("out", [G, C], F32, kind="ExternalOutput")

    # internal DRAM: per-chunk partial tables and RS outputs
    z_part = [
        nc.dram_tensor(f"z_part{k}", [NC * CHK_ROWS[k], H], TDT)
        for k in range(NKCH)
    ]
    z_own = [
        nc.dram_tensor(f"z_own{k}", [CHK_ROWS[k], H], TDT)
        for k in range(NKCH)
    ]
    pp = nc.dram_tensor("pp", [G, H], F32)
    pooled = nc.dram_tensor("pooled", [G, H], F32, addr_space="Shared")

    rg = [list(range(NC))]

    from contextlib import ExitStack
    ctx = ExitStack()
    with tile.TileContext(nc) as tc, ctx:
        cpool = ctx.enter_context(tc.tile_pool(name="consts", bufs=1))
        MSG_BUFS = int(os.environ.get("KGCN_MSG_BUFS", "11"))
        msgp = ctx.enter_context(tc.tile_pool(name="msg", bufs=MSG_BUFS))
        SEL_BUFS = int(os.environ.get("KGCN_SEL_BUFS", "4"))
        selp = ctx.enter_context(tc.tile_pool(name="sel", bufs=SEL_BUFS))
        wrk = ctx.enter_context(tc.tile_pool(name="wrk", bufs=4))
        zldp = ctx.enter_context(tc.tile_pool(name="zld", bufs=2))
        ps_a = ctx.enter_context(tc.tile_pool(name="psA", bufs=1, space="PSUM"))
        ps_b = ctx.enter_context(tc.tile_pool(name="psB", bufs=4, space="PSUM"))
        ps_c = ctx.enter_context(tc.tile_pool(name="psC", bufs=1, space="PSUM"))

        def load_const(name, shape, dt):
            t = cpool.tile(shape, dt, tag=name)
            nc.sync.dma_start(out=t[:], in_=din[name][:])
            return t

        gidx_sb = load_const("gidx", [128, TOKP // 16], I16)
        dloc_sb = load_const("dloc", [128, NPAIR], TDT)
        dinv_sb = load_const("dinv", [128, NB], F32)
        acol_sb = load_const("acol", [128, NB], F32)
        batc_sb = load_const("batchf", [128, NB], F32)
        w_sb = {1: load_const("W1", [H, H], TDT), 2: load_const("W2", [H, H], TDT)}
        wp_sb = load_const("Wp", [H, C], F32)
        w0r_sb = load_const("W0r", [128, H], F32)
        br_sb = {
            0: load_const("b0r", [128, H], F32),
            1: load_const("b1r", [128, H], F32),
            2: load_const("b2r", [128, H], F32),
        }
        bpr_sb = load_const("bpr", [128, C], F32)
        id_sb = load_const("ident", [128, 128], F32)
        idt_sb = load_const("identt", [128, 128], TDT)
        iot_sb = load_const("iotar", [128, 128], TDT)
        gio_sb = load_const("giota", [128, G], F32)
        ivc_sb = load_const("invc", [128, 4], F32)

        x_sb = cpool.tile([128, S], TDT, tag="x")
        y_tiles = [cpool.tile([128, S], TDT, tag="y0", name="ytile0")] * 2

        def xblk(J):
            return x_sb[:, J * 128:(J + 1) * 128]

        # ---- layer 0: x1 = relu(a * W0 + b0) ----
        for J in range(NB):
            t0 = wrk.tile([128, H], F32, tag="l0")
            nc.vector.scalar_tensor_tensor(
                out=t0[:],
                in0=w0r_sb[:],
                scalar=acol_sb[:, J:J + 1],
                in1=br_sb[0][:],
                op0=mybir.AluOpType.mult,
                op1=mybir.AluOpType.add,
            )
            nc.scalar.activation(xblk(J), t0[:], mybir.ActivationFunctionType.Relu)

        # ---- conv layers ----
        pool_ps = ps_c.tile([128, 4 * H], F32, tag="pool", name="poolacc")
        for layer in range(1, NLAYER + 1):
            y_sb = y_tiles[layer % 2]

            # phase A: y = dinv * (x @ W) into SBUF
            for J in range(NB):
                xt_ps = ps_a.tile([128, 128], TDT, tag="xt")
                nc.tensor.transpose(out=xt_ps[:], in_=xblk(J), identity=idt_sb[:])
                xt_sb = wrk.tile([128, 128], TDT, tag="xt_sb")
                nc.scalar.copy(xt_sb[:], xt_ps[:])
                h_ps = ps_a.tile([128, H], F32, tag="h")
                nc.tensor.matmul(
                    out=h_ps[:], lhsT=xt_sb[:], rhs=w_sb[layer][:],
                    start=True, stop=True,
                )
                nc.scalar.mul(
                    y_sb[:, J * 128:(J + 1) * 128], h_ps[:],
                    mul=dinv_sb[:, J:J + 1],
                )

            # phase C for one chunk: x' = relu(dinv * (z + y_own) + b);
            # on the last layer the pooling matmuls are folded in as well
            def phase_c(k):
                if NO_RS:
                    return
                nc.gpsimd.collective_compute(
                    "ReduceScatter",
                    mybir.AluOpType.add,
                    replica_groups=rg,
                    ins=[z_part[k][:]],
                    outs=[z_own[k][:]],
                )
                zk = zldp.tile([128, CHK_BLKS[k] * 128], TDT, tag="zk",
                               name=f"zk{k}_{layer}")
                nc.sync.dma_start(
                    out=zk[:].rearrange("p (j e) -> p j e", e=H),
                    in_=z_own[k][:].rearrange("(j p) e -> p j e", p=128),
                )
                for jj in range(CHK_BLKS[k]):
                    J = CHK_J0[k] + jj
                    # self-loop term: z_total = z_gathered + y_own
                    t0 = wrk.tile([128, H], F32, tag="pz")
                    nc.vector.tensor_tensor(
                        out=t0[:],
                        in0=zk[:, jj * 128:(jj + 1) * 128],
                        in1=y_sb[:, J * 128:(J + 1) * 128],
                        op=mybir.AluOpType.add,
                    )
                    t1 = wrk.tile([128, H], F32, tag="pc")
                    nc.vector.scalar_tensor_tensor(
                        out=t1[:],
                        in0=t0[:],
                        scalar=dinv_sb[:, J:J + 1],
                        in1=br_sb[layer][:],
                        op0=mybir.AluOpType.mult,
                        op1=mybir.AluOpType.add,
                    )
                    nc.scalar.activation(
                        xblk(J), t1[:], mybir.ActivationFunctionType.Relu
                    )
                    if layer == NLAYER:
                        selg = wrk.tile([128, G], TDT, tag="selg")
                        nc.vector.tensor_tensor(
                            out=selg[:],
                            in0=batc_sb[:, J:J + 1].to_broadcast([128, G]),
                            in1=gio_sb[:],
                            op=mybir.AluOpType.is_equal,
                        )
                        for gb in range(4):
                            nc.tensor.matmul(
                                out=pool_ps[:, gb * H:(gb + 1) * H],
                                lhsT=selg[:, gb * 128:(gb + 1) * 128],
                                rhs=xblk(J),
                                start=(J == 0),
                                stop=(J == NB - 1),
                            )

            # phase B: gather + one-hot aggregation + chunked ReduceScatter
            zlive = {}
            evct = 0
            cur_k = -1
            for gi, (k, gt0, gn) in enumerate(gathers):
                if k != cur_k:
                    if cur_k >= 0:
                        phase_c(cur_k)
                    cur_k = k
                nslots = gn // 128
                mt = msgp.tile([128, nslots * H], TDT, tag="msg")
                sbuf_gather_notrans(
                    nc,
                    out_ap=mt[:].rearrange("p (s e) -> p s e", e=H),
                    in_ap=y_sb[:].rearrange("p (b e) -> p b e", e=H),
                    idxs_ap=gidx_sb[:, gt0 // 16:(gt0 + gn) // 16],
                    num_idxs=gn,
                    elem_size=H,
                    queue_num=gi % 4,
                )
                p0, p1 = gather_pair0[gi], gather_pair0[gi + 1]
                npg = p1 - p0
                st = selp.tile([128, npg * 128], TDT, tag="sel")
                if not NO_SEL:
                    nc.vector.tensor_tensor(
                        out=st[:].rearrange("p (s e) -> p s e", e=128),
                        in0=dloc_sb[:, p0:p1, None].to_broadcast([128, npg, 128]),
                        in1=iot_sb[:, None, :].to_broadcast([128, npg, 128]),
                        op=mybir.AluOpType.is_equal,
                    )
                for pi in range(p0, p1) if not NO_MM else []:
                    s, B = pairs[pi]
                    if pfirst[pi]:
                        zlive[B] = ps_b.tile([128, H], F32, tag="z",
                                             name=f"zacc{B}_{layer}")
                    sl = s - gt0 // 128
                    nc.tensor.matmul(
                        out=zlive[B][:],
                        lhsT=st[:, (pi - p0) * 128:(pi - p0 + 1) * 128],
                        rhs=mt[:].rearrange("p (s e) -> p s e", e=H)[:, sl, :],
                        start=bool(pfirst[pi]),
                        stop=bool(plast[pi]),
                    )
                    if plast[pi]:
                        t1 = wrk.tile([128, H], TDT, tag="ev")
                        if evct % 2 == 0:
                            nc.scalar.copy(t1[:], zlive[B][:])
                        else:
                            nc.vector.tensor_copy(t1[:], zlive[B][:])
                        evct += 1
                        rb = int(ev_row[B])
                        nc.sync.dma_start(
                            out=z_part[int(ev_k[B])][rb:rb + 128, :], in_=t1[:]
                        )
                        del zlive[B]
            phase_c(cur_k)
            assert not zlive

        # ---- pooling eviction ----
        for gb in range(4):
            t2 = wrk.tile([128, H], F32, tag="ppev")
            nc.scalar.copy(t2[:], pool_ps[:, gb * H:(gb + 1) * H])
            nc.sync.dma_start(out=pp[gb * 128:(gb + 1) * 128, :], in_=t2[:])
        nc.gpsimd.collective_compute(
            "AllReduce",
            mybir.AluOpType.add,
            replica_groups=rg,
            ins=[pp[:]],
            outs=[pooled[:]],
        )

        # ---- head + log_softmax ----
        for gb in range(4):
            pl = wrk.tile([128, H], F32, tag="pl")
            nc.sync.dma_start(out=pl[:], in_=pooled[gb * 128:(gb + 1) * 128, :])
            plm = wrk.tile([128, H], F32, tag="plm")
            nc.scalar.mul(plm[:], pl[:], mul=ivc_sb[:, gb:gb + 1])
            pt_ps = ps_a.tile([128, 128], F32, tag="xt")
            nc.tensor.transpose(out=pt_ps[:], in_=plm[:], identity=id_sb[:])
            pt_sb = wrk.tile([128, 128], F32, tag="pts")
            nc.scalar.copy(pt_sb[:], pt_ps[:])
            lg_ps = ps_a.tile([128, C], F32, tag="h")
            nc.tensor.matmul(
                out=lg_ps[:], lhsT=pt_sb[:], rhs=wp_sb[:], start=True, stop=True
            )
            tl = wrk.tile([128, C], F32, tag="tl")
            nc.vector.tensor_tensor(
                out=tl[:], in0=lg_ps[:], in1=bpr_sb[:], op=mybir.AluOpType.add
            )
            mx = wrk.tile([128, 1], F32, tag="mx")
            nc.vector.tensor_reduce(
                out=mx[:], in_=tl[:], axis=mybir.AxisListType.X,
                op=mybir.AluOpType.max,
            )
            nmx = wrk.tile([128, 1], F32, tag="nmx")
            nc.vector.tensor_scalar_mul(nmx[:], mx[:], -1.0)
            ex = wrk.tile([128, C], F32, tag="ex")
            ssum = wrk.tile([128, 1], F32, tag="ssum")
            nc.scalar.activation(
                ex[:], tl[:], mybir.ActivationFunctionType.Exp,
                bias=nmx[:, :1], accum_out=ssum[:],
            )
            lns = wrk.tile([128, 1], F32, tag="lns")
            nc.scalar.activation(lns[:], ssum[:], mybir.ActivationFunctionType.Ln)
            ofs = wrk.tile([128, 1], F32, tag="ofs")
            nc.vector.tensor_tensor(
                out=ofs[:], in0=nmx[:], in1=lns[:], op=mybir.AluOpType.subtract
            )
            fin = wrk.tile([128, C], F32, tag="fin")
            nc.vector.tensor_scalar_add(fin[:], tl[:], ofs[:, :1])
            nc.sync.dma_start(out=out[gb * 128:(gb + 1) * 128, :], in_=fin[:])

    nc.compile()
    return nc


_CACHE = {}


def kernel(edge_index, batch, W0, b0, W1, b1, W2, b2, Wp, bp):
    edge_index = np.asarray(edge_index, dtype=np.int32)
    batch = np.asarray(batch, dtype=np.int32)
    W0 = np.asarray(W0, dtype=np.float32)
    b0 = np.asarray(b0, dtype=np.float32)
    W1 = np.asarray(W1, dtype=np.float32)
    b1 = np.asarray(b1, dtype=np.float32)
    W2 = np.asarray(W2, dtype=np.float32)
    b2 = np.asarray(b2, dtype=np.float32)
    Wp = np.asarray(Wp, dtype=np.float32)
    bp = np.asarray(bp, dtype=np.float32)

    key = hash((edge_index.tobytes(), batch.tobytes()))
    if key not in _CACHE:
        sched, percore, invcnt = preprocess(edge_index, batch)
        nc = build_program(sched)
        _CACHE[key] = (sched, percore, invcnt, nc)
    sched, percore, invcnt, nc = _CACHE[key]

    consts = {
        "W1": W1.astype(NP_TDT),
        "W2": W2.astype(NP_TDT),
        "Wp": Wp,
        "W0r": np.tile(W0.reshape(1, H), (128, 1)),
        "b0r": np.tile(b0.reshape(1, H), (128, 1)),
        "b1r": np.tile(b1.reshape(1, H), (128, 1)),
        "b2r": np.tile(b2.reshape(1, H), (128, 1)),
        "bpr": np.tile(bp.reshape(1, C), (128, 1)),
        "ident": np.eye(128, dtype=np.float32),
        "identt": np.eye(128, dtype=np.float32).astype(NP_TDT),
        "iotar": np.tile(
            np.arange(128, dtype=np.float32).astype(NP_TDT).reshape(1, 128),
            (128, 1),
        ),
        "giota": np.tile(np.arange(G, dtype=np.float32).reshape(1, G), (128, 1)),
        "invc": invcnt,
    }
    consts = {k: np.ascontiguousarray(v) for k, v in consts.items()}

    in_maps = []
    for c in range(NC):
        m = {
            "gidx": percore["gidx"][c],
            "dloc": percore["dloc"][c],
            "dinv": percore["dinv_pj"][c],
            "acol": percore["a_pj"][c],
            "batchf": percore["batc_pj"][c],
        }
        m.update(consts)
        in_maps.append(m)

    import os
    trace = bool(int(os.environ.get("KGCN_TRACE", "0")))
    res = run_bass_kernel_spmd(
        nc, in_maps, core_ids=list(range(NC)), trace=trace
    )
    kernel.last_results = res
    return res.results[0]["out"]
